# revision 1
# baseline (speedup 1.0000x reference)
"""MedianGCNAggregator Trainium2 kernel.

out = relu(median_over_{self,32 neighbors}(features) @ kernel + bias)

Strategy: data-parallel over nodes across 8 cores. Per core, nodes stream
through SBUF in groups of T*128; each group's 33 candidate rows (32
neighbors + self) live as "wire" rows of shape [128 node_partition, T, 128
chan]. The median per (node, channel) is an exact rank-16-of-33 min/max
selection network (153 comparators built from two Green 16-sorters, a
liveness-pruned Batcher merge, and a 2-op insert of the self row; BDD-
verified equal to threshold-17-of-33). Network ops are fused into strided
multi-row TT instructions where comparator patterns are uniform. The median
then takes a PE transpose, a 128x128 matmul, fused bias+relu on ScalarE,
and streams back to DRAM. The vector engine is the bottleneck; DMA, PE and
ScalarE run far below it.
"""
import numpy as np

# fmt: off
_NET_BATCHES = [(0,0,2,1,2,32,1,16),(1,0,2,1,2,1,2,16),(0,1,4,3,4,16,2,8),(0,32,2,33,2,0,2,8),(1,1,4,3,4,3,4,8),(1,32,2,33,2,33,2,8),(0,0,3,2,5,17,4,2),(0,4,4,6,4,25,4,2),(0,11,1,15,1,32,1,1),(0,12,4,14,4,34,2,2),(0,19,1,23,1,38,1,1),(0,20,4,22,4,40,2,2),(0,27,1,31,1,44,1,1),(0,28,1,30,1,46,1,1),(0,33,4,35,4,1,4,4),(1,0,3,2,5,2,5,2),(1,4,4,6,4,6,4,2),(1,11,1,15,1,15,1,1),(1,12,4,14,4,14,4,2),(1,19,1,23,1,23,1,1),(1,20,4,22,4,22,4,2),(1,27,1,31,1,31,1,1),(1,28,1,30,1,30,1,1),(1,33,4,35,4,35,4,4),(0,1,1,5,1,12,4,2),(0,7,1,15,1,19,1,1),(0,9,1,13,1,20,4,2),(0,17,1,25,1,27,1,1),(0,18,1,22,1,28,1,1),(0,21,1,32,1,33,1,1),(0,23,1,31,1,37,1,1),(0,26,1,30,1,41,1,1),(0,29,1,34,1,45,1,1),(0,35,1,39,1,0,3,2),(0,38,4,44,2,4,4,2),(0,43,1,47,1,11,1,1),(1,1,1,5,1,5,1,2),(1,7,1,15,1,15,1,1),(1,9,1,13,1,13,1,2),(1,17,1,25,1,25,1,1),(1,18,1,22,1,22,1,1),(1,21,1,32,1,32,1,1),(1,23,1,31,1,31,1,1),(1,26,1,30,1,30,1,1),(1,29,1,34,1,34,1,1),(1,35,1,39,1,39,1,2),(1,38,4,44,2,44,2,2),(1,43,1,47,1,47,1,1),(0,0,1,40,1,9,1,1),(0,3,1,12,2,10,7,2),(0,8,1,20,1,18,1,1),(0,11,1,46,1,21,1,1),(0,15,1,31,1,23,1,1),(0,16,1,25,1,26,1,1),(0,19,1,32,1,29,1,1),(0,22,1,39,1,35,1,1),(0,24,1,34,1,36,1,1),(0,28,1,5,1,38,1,1),(0,30,1,47,1,42,1,1),(0,33,1,6,1,1,1,1),(0,37,1,44,1,2,1,1),(0,41,1,13,1,7,1,1),(1,0,1,40,1,40,1,1),(1,3,1,12,2,12,2,2),(1,8,1,20,1,20,1,1),(1,11,1,46,1,46,1,1),(1,16,1,25,1,25,1,1),(1,19,3,32,7,32,7,2),(1,24,3,34,11,34,11,2),(1,28,1,5,1,5,1,1),(1,30,1,47,1,47,1,1),(1,33,1,6,1,6,1,1),(1,37,1,44,1,44,1,1),(1,41,1,13,1,13,1,1),(0,2,1,42,1,15,1,1),(0,7,3,21,5,16,3,2),(0,12,1,25,1,22,1,1),(0,18,1,36,1,24,1,1),(0,20,9,34,1,27,1,2),(0,32,1,39,1,0,1,1),(0,38,1,9,1,3,1,1),(0,40,1,5,1,4,1,1),(0,44,1,47,1,8,1,1),(0,46,1,13,1,11,1,1),(1,2,1,42,1,42,1,1),(1,7,3,21,5,21,5,2),(1,12,1,25,1,25,1,1),(1,18,1,36,1,36,1,1),(1,20,9,34,1,34,1,2),(1,32,1,39,1,39,1,1),(1,38,1,9,1,9,1,1),(1,40,1,5,1,5,1,1),(1,44,1,47,1,47,1,1),(1,46,1,13,1,13,1,1),(0,0,1,35,1,7,1,1),(0,1,1,25,1,10,1,1),(0,8,1,42,1,12,1,1),(0,15,1,14,1,18,1,1),(0,17,1,34,1,20,1,1),(0,22,1,26,1,29,1,1),(0,27,1,36,1,30,1,1),(0,28,1,6,1,31,1,1),(0,39,1,47,1,2,1,1),(1,0,1,35,1,35,1,1),(1,1,1,25,1,25,1,1),(1,8,1,42,1,42,1,1),(1,15,1,14,1,14,1,1),(1,17,1,34,1,34,1,1),(1,19,3,24,2,24,2,2),(1,27,1,36,1,36,1,1),(1,28,1,6,1,6,1,1),(0,5,4,6,19,1,7,2),(0,10,3,3,11,15,2,2),(0,18,2,11,5,19,3,2),(0,21,1,34,1,27,1,1),(0,31,1,4,1,28,1,1),(0,35,1,42,1,0,1,1),(1,5,4,6,19,6,19,2),(1,10,3,3,11,3,11,2),(1,18,2,11,5,11,5,2),(1,21,1,34,1,34,1,1),(1,29,1,30,1,30,1,1),(1,31,1,4,1,4,1,1),(0,3,1,8,1,5,1,1),(0,4,3,1,5,9,1,2),(0,11,1,17,1,13,1,1),(0,12,3,14,12,18,2,2),(0,16,1,27,1,21,1,1),(0,19,3,34,2,29,2,2),(0,28,1,25,1,32,1,1),(1,3,1,8,1,8,1,1),(1,4,3,1,5,1,5,2),(1,11,1,17,1,17,1,1),(1,12,3,14,12,14,12,2),(1,16,1,27,1,27,1,1),(1,19,3,34,2,34,2,2),(1,28,1,25,1,25,1,1),(0,1,1,17,1,4,1,1),(0,6,1,14,1,7,1,1),(0,8,1,32,1,11,1,1),(0,10,1,18,1,12,1,1),(0,25,2,9,20,15,1,2),(0,34,1,13,1,3,1,1),(1,5,1,21,1,21,1,1),(1,8,1,32,1,32,1,1),(1,20,1,31,1,31,1,1),(1,25,1,9,1,9,1,1),(1,26,1,36,1,36,1,1),(1,27,1,29,1,29,1,1),(1,34,1,13,1,13,1,1),(0,0,1,21,1,1,1,1),(0,7,1,36,1,5,1,1),(0,9,1,13,1,6,1,1),(0,15,1,3,1,8,1,1),(1,4,1,30,1,30,1,1),(1,11,1,16,15,16,15,2),(1,32,1,29,1,29,1,1),(0,2,21,16,13,0,3,2),(1,6,1,24,1,24,1,1),(1,8,1,45,1,45,1,1),(0,0,3,30,1,2,2,2),(1,1,1,24,1,24,1,1),(1,5,1,45,1,45,1,1),(0,4,1,24,1,0,1,1),(1,2,1,45,1,45,1,1),(0,0,1,45,1,1,1,1),(1,0,1,45,3,45,3,2),(0,45,1,48,1,0,1,1)]
_NET_NROWS = 49
_NET_OUTROW = 0
_NET_XROW = 48
# fmt: on

N, S, F, U = 100000, 32, 128, 128
NCORES = 8
T = 4                       # node-subtiles per group
GROUP = T * 128             # nodes per group
NC_NODES = 12800            # padded nodes per core (NGROUPS * GROUP)
NGROUPS = NC_NODES // GROUP
N_STAGE = 32                # rows 0..31 live in the staging tile
N_SCRATCH = _NET_NROWS - N_STAGE


def _build_program():
    from concourse import bacc, mybir
    from concourse.tile import TileContext
    from concourse.masks import make_identity

    f32 = mybir.dt.float32

    nc = bacc.Bacc(None, target_bir_lowering=False)
    x_d = nc.declare_dram_parameter("x", [NC_NODES, F], f32, isOutput=False)
    g_d = nc.declare_dram_parameter("neigh", [NC_NODES, S, F], f32, isOutput=False)
    k_d = nc.declare_dram_parameter("kern", [F, U], f32, isOutput=False)
    b_d = nc.declare_dram_parameter("bias", [U, 1], f32, isOutput=False)
    o_d = nc.declare_dram_parameter("out", [U, NC_NODES], f32, isOutput=True)

    with TileContext(nc) as tc:
        with (
            tc.tile_pool(name="consts", bufs=1) as consts,
            tc.tile_pool(name="stage", bufs=2) as stage_pool,
            tc.tile_pool(name="scratch", bufs=1) as scratch_pool,
            tc.tile_pool(name="medt", bufs=2) as medt_pool,
            tc.tile_pool(name="outs", bufs=2) as out_pool,
            tc.tile_pool(name="touch", bufs=2) as touch_pool,
            tc.tile_pool(name="ps_tr", bufs=2, space="PSUM") as ps_tr,
            tc.tile_pool(name="ps_mm", bufs=2, space="PSUM") as ps_mm,
        ):
            kern_sb = consts.tile([F, U], f32)
            nc.sync.dma_start(out=kern_sb, in_=k_d[:])
            bias_sb = consts.tile([U, 1], f32)
            nc.sync.dma_start(out=bias_sb, in_=b_d[:])
            ident = consts.tile([128, 128], f32)
            make_identity(nc, ident)

            # scratch is DVE-private (plus the x DMA) and serial on DVE, so a
            # single buffer suffices; staging/outs double-buffer the DMA.
            scratch = scratch_pool.tile([128, N_SCRATCH, T, F], f32)

            for g in range(NGROUPS):
                base = g * GROUP
                staging = stage_pool.tile([128, T, S, F], f32)
                nc.sync.dma_start(
                    out=staging[:],
                    in_=g_d[base : base + GROUP].rearrange(
                        "(t p) s c -> p t s c", p=128
                    ),
                )
                # self row -> dedicated scratch row (read only by the final
                # insert ops, so the single-buffered slot never stalls DMA)
                nc.sync.dma_start(
                    out=scratch[:, _NET_XROW - N_STAGE],
                    in_=x_d[base : base + GROUP].rearrange("(t p) c -> p t c", p=128),
                )

                # Absorb the two DMA-queue waits into DVE's clock via tiny
                # copies, so network TT ops never need >1 sync wait (HW limit).
                touch = touch_pool.tile([128, 2], f32)
                nc.vector.tensor_copy(
                    out=touch[:, 0:1], in_=scratch[:, _NET_XROW - N_STAGE, 0, 0:1]
                )
                nc.vector.tensor_copy(out=touch[:, 1:2], in_=staging[:, 0, 0, 0:1])

                def run_ap(row, step, count):
                    assert step > 0 or count == 1
                    step = max(step, 1)
                    last = row + step * (count - 1)
                    if row < N_STAGE:
                        assert last < N_STAGE
                        return staging[:, :, row : last + 1 : step, :].rearrange(
                            "p t r f -> p r t f"
                        )
                    r0 = row - N_STAGE
                    r1 = last - N_STAGE
                    assert 0 <= r0 and r1 < N_SCRATCH
                    return scratch[:, r0 : r1 + 1 : step]

                for (is_max, a, da, b, db, d, dd, cnt) in _NET_BATCHES:
                    nc.vector.tensor_tensor(
                        out=run_ap(d, dd, cnt),
                        in0=run_ap(a, da, cnt),
                        in1=run_ap(b, db, cnt),
                        op=mybir.AluOpType.max if is_max else mybir.AluOpType.min,
                    )

                med = run_ap(_NET_OUTROW, 1, 1)  # [128, 1|, T, F]-ish view
                medt = medt_pool.tile([128, T, F], f32)
                for t in range(T):
                    trp = ps_tr.tile([128, 128], f32)
                    nc.tensor.transpose(trp, med[:, 0, t, :] if med.shape[1] == 1 else med[:, t, :], ident)
                    nc.scalar.copy(out=medt[:, t, :], in_=trp)
                medt_flat = medt[:].rearrange("p t n -> p (t n)")
                out_sb = out_pool.tile([128, GROUP], f32)
                n_chunks = -(-GROUP // 512)
                chunk = GROUP // n_chunks
                for ci in range(n_chunks):
                    off = ci * chunk
                    out_ps = ps_mm.tile([128, chunk], f32)
                    nc.tensor.matmul(
                        out_ps,
                        lhsT=kern_sb[:],
                        rhs=medt_flat[:, off : off + chunk],
                        start=True,
                        stop=True,
                    )
                    nc.scalar.activation(
                        out=out_sb[:, off : off + chunk],
                        in_=out_ps,
                        func=mybir.ActivationFunctionType.Relu,
                        bias=bias_sb[:],
                        scale=1.0,
                    )
                nc.sync.dma_start(out=o_d[:, base : base + GROUP], in_=out_sb)
    nc.finalize()
    return nc


def kernel(x, neigh_x, kernel, bias):
    from concourse.bass_utils import run_bass_kernel_spmd

    x = np.ascontiguousarray(x, dtype=np.float32)
    neigh_x = np.ascontiguousarray(neigh_x, dtype=np.float32)
    kern = np.ascontiguousarray(kernel, dtype=np.float32)
    bias2 = np.ascontiguousarray(bias, dtype=np.float32).reshape(U, 1)

    per = N // NCORES  # 12500
    x_pad = np.zeros((NCORES, NC_NODES, F), dtype=np.float32)
    g_pad = np.zeros((NCORES, NC_NODES, S, F), dtype=np.float32)
    for c in range(NCORES):
        x_pad[c, :per] = x[c * per : (c + 1) * per]
        g_pad[c, :per] = neigh_x[c * per : (c + 1) * per]

    nc = _build_program()
    in_maps = [
        {"x": x_pad[c], "neigh": g_pad[c], "kern": kern, "bias": bias2}
        for c in range(NCORES)
    ]
    res = run_bass_kernel_spmd(nc, in_maps, list(range(NCORES)))
    out = np.empty((N, U), dtype=np.float32)
    for c in range(NCORES):
        out[c * per : (c + 1) * per] = res.results[c]["out"][:, :per].T
    return out



# revision 25
# speedup vs baseline: 1.9689x; 1.9689x over previous
"""MedianGCNAggregator Trainium2 kernel.

out = relu(median_over_{self,32 neighbors}(features) @ kernel + bias)

Strategy: data-parallel over nodes across 8 cores. Per core, nodes stream
through SBUF in groups of T*128; the median per (node, channel) is an exact
rank-16-of-33 min/max selection network (153 comparators built from two
Green 16-sorters, a liveness-pruned Batcher merge, and a 2-op insert of the
self row; verified equal to np.sort rank-16 on random and tied inputs).

The network runs on the Vector engine in fp16, where packed 16-bit
tensor_tensor ops hit the DVE 2x_1p fast path (2 elem/cycle/partition,
measured 0.52 ns/elem vs 1.04 fp32). The Scalar engine converts the fp32
DMA-landed inputs to fp16 ahead of the network; layer 1 is the only
consumer of the converted input rows, so the single-buffered fp16 input
tile frees for the next group's conversion two instructions into each
group. The median then takes a PE transpose, a 128x128 fp16 matmul, fused
bias+relu on ScalarE, and streams back to DRAM. DVE remains the bottleneck;
DMA, PE and ScalarE run far below it.
"""
import numpy as np

# fmt: off
_NET_BATCHES = [(0,0,2,1,2,32,1,16),(1,0,2,1,2,1,2,16),(0,1,4,3,4,16,2,8),(0,32,2,33,2,0,2,8),(1,1,4,3,4,3,4,8),(1,32,2,33,2,33,2,8),(0,0,3,2,5,17,4,2),(0,4,4,6,4,25,4,2),(0,11,1,15,1,32,1,1),(0,12,4,14,4,34,2,2),(0,19,1,23,1,38,1,1),(0,20,4,22,4,40,2,2),(0,27,1,31,1,44,1,1),(0,28,1,30,1,46,1,1),(0,33,4,35,4,1,4,4),(1,0,3,2,5,2,5,2),(1,4,4,6,4,6,4,2),(1,11,1,15,1,15,1,1),(1,12,4,14,4,14,4,2),(1,19,1,23,1,23,1,1),(1,20,4,22,4,22,4,2),(1,27,1,31,1,31,1,1),(1,28,1,30,1,30,1,1),(1,33,4,35,4,35,4,4),(0,1,1,5,1,12,4,2),(0,7,1,15,1,19,1,1),(0,9,1,13,1,20,4,2),(0,17,1,25,1,27,1,1),(0,18,1,22,1,28,1,1),(0,21,1,32,1,33,1,1),(0,23,1,31,1,37,1,1),(0,26,1,30,1,41,1,1),(0,29,1,34,1,45,1,1),(0,35,1,39,1,0,3,2),(0,38,4,44,2,4,4,2),(0,43,1,47,1,11,1,1),(1,1,1,5,1,5,1,2),(1,7,1,15,1,15,1,1),(1,9,1,13,1,13,1,2),(1,17,1,25,1,25,1,1),(1,18,1,22,1,22,1,1),(1,21,1,32,1,32,1,1),(1,23,1,31,1,31,1,1),(1,26,1,30,1,30,1,1),(1,29,1,34,1,34,1,1),(1,35,1,39,1,39,1,2),(1,38,4,44,2,44,2,2),(1,43,1,47,1,47,1,1),(0,0,1,40,1,9,1,1),(0,3,1,12,2,10,7,2),(0,8,1,20,1,18,1,1),(0,11,1,46,1,21,1,1),(0,15,1,31,1,23,1,1),(0,16,1,25,1,26,1,1),(0,19,1,32,1,29,1,1),(0,22,1,39,1,35,1,1),(0,24,1,34,1,36,1,1),(0,28,1,5,1,38,1,1),(0,30,1,47,1,42,1,1),(0,33,1,6,1,1,1,1),(0,37,1,44,1,2,1,1),(0,41,1,13,1,7,1,1),(1,0,1,40,1,40,1,1),(1,3,1,12,2,12,2,2),(1,8,1,20,1,20,1,1),(1,11,1,46,1,46,1,1),(1,16,1,25,1,25,1,1),(1,19,3,32,7,32,7,2),(1,24,3,34,11,34,11,2),(1,28,1,5,1,5,1,1),(1,30,1,47,1,47,1,1),(1,33,1,6,1,6,1,1),(1,37,1,44,1,44,1,1),(1,41,1,13,1,13,1,1),(0,2,1,42,1,15,1,1),(0,7,3,21,5,16,3,2),(0,12,1,25,1,22,1,1),(0,18,1,36,1,24,1,1),(0,20,9,34,1,27,1,2),(0,32,1,39,1,0,1,1),(0,38,1,9,1,3,1,1),(0,40,1,5,1,4,1,1),(0,44,1,47,1,8,1,1),(0,46,1,13,1,11,1,1),(1,2,1,42,1,42,1,1),(1,7,3,21,5,21,5,2),(1,12,1,25,1,25,1,1),(1,18,1,36,1,36,1,1),(1,20,9,34,1,34,1,2),(1,32,1,39,1,39,1,1),(1,38,1,9,1,9,1,1),(1,40,1,5,1,5,1,1),(1,44,1,47,1,47,1,1),(1,46,1,13,1,13,1,1),(0,0,1,35,1,7,1,1),(0,1,1,25,1,10,1,1),(0,8,1,42,1,12,1,1),(0,15,1,14,1,18,1,1),(0,17,1,34,1,20,1,1),(0,22,1,26,1,29,1,1),(0,27,1,36,1,30,1,1),(0,28,1,6,1,31,1,1),(0,39,1,47,1,2,1,1),(1,0,1,35,1,35,1,1),(1,1,1,25,1,25,1,1),(1,8,1,42,1,42,1,1),(1,15,1,14,1,14,1,1),(1,17,1,34,1,34,1,1),(1,19,3,24,2,24,2,2),(1,27,1,36,1,36,1,1),(1,28,1,6,1,6,1,1),(0,5,4,6,19,1,7,2),(0,10,3,3,11,15,2,2),(0,18,2,11,5,19,3,2),(0,21,1,34,1,27,1,1),(0,31,1,4,1,28,1,1),(0,35,1,42,1,0,1,1),(1,5,4,6,19,6,19,2),(1,10,3,3,11,3,11,2),(1,18,2,11,5,11,5,2),(1,21,1,34,1,34,1,1),(1,29,1,30,1,30,1,1),(1,31,1,4,1,4,1,1),(0,3,1,8,1,5,1,1),(0,4,3,1,5,9,1,2),(0,11,1,17,1,13,1,1),(0,12,3,14,12,18,2,2),(0,16,1,27,1,21,1,1),(0,19,3,34,2,29,2,2),(0,28,1,25,1,32,1,1),(1,3,1,8,1,8,1,1),(1,4,3,1,5,1,5,2),(1,11,1,17,1,17,1,1),(1,12,3,14,12,14,12,2),(1,16,1,27,1,27,1,1),(1,19,3,34,2,34,2,2),(1,28,1,25,1,25,1,1),(0,1,1,17,1,4,1,1),(0,6,1,14,1,7,1,1),(0,8,1,32,1,11,1,1),(0,10,1,18,1,12,1,1),(0,25,2,9,20,15,1,2),(0,34,1,13,1,3,1,1),(1,5,1,21,1,21,1,1),(1,8,1,32,1,32,1,1),(1,20,1,31,1,31,1,1),(1,25,1,9,1,9,1,1),(1,26,1,36,1,36,1,1),(1,27,1,29,1,29,1,1),(1,34,1,13,1,13,1,1),(0,0,1,21,1,1,1,1),(0,7,1,36,1,5,1,1),(0,9,1,13,1,6,1,1),(0,15,1,3,1,8,1,1),(1,4,1,30,1,30,1,1),(1,11,1,16,15,16,15,2),(1,32,1,29,1,29,1,1),(0,2,21,16,13,0,3,2),(1,6,1,24,1,24,1,1),(1,8,1,45,1,45,1,1),(0,0,3,30,1,2,2,2),(1,1,1,24,1,24,1,1),(1,5,1,45,1,45,1,1),(0,4,1,24,1,0,1,1),(1,2,1,45,1,45,1,1),(0,0,1,45,1,1,1,1),(1,0,1,45,1,45,1,1),(1,1,1,48,1,47,1,1),(0,45,1,47,1,0,1,1)]
_NET_NROWS = 49
_NET_OUTROW = 0
_NET_XROW = 48
# fmt: on

N, S, F, U = 100000, 32, 128, 128
NCORES = 8
FP32_TAIL = False           # debug: run transpose/matmul in fp32
DEBUG_ROW = None            # debug: bypass network, output relu(row @ kernel)
NET_LIMIT = None            # debug: emit only the first NET_LIMIT batches
DEBUG_SRC = None            # debug: med_buf source ('in', r) | ('x',) | ('scratch', r)
T = 7                       # node-subtiles per group
GROUP = T * 128             # nodes per group
NC_NODES = 12544            # padded nodes per core (NGROUPS * GROUP)
NGROUPS = NC_NODES // GROUP


def _build_program():
    from concourse import bacc, mybir
    from concourse.tile import TileContext
    from concourse.masks import make_identity

    f32 = mybir.dt.float32
    f16 = mybir.dt.float16

    nc = bacc.Bacc(None, target_bir_lowering=False)
    x_d = nc.declare_dram_parameter("x", [NC_NODES, F], f32, isOutput=False)
    g_d = nc.declare_dram_parameter("neigh", [NC_NODES, S, F], f32, isOutput=False)
    k_d = nc.declare_dram_parameter("kern", [F, U], f32, isOutput=False)
    b_d = nc.declare_dram_parameter("bias", [U, 1], f32, isOutput=False)
    o_d = nc.declare_dram_parameter("out", [U, NC_NODES], f32, isOutput=True)

    with TileContext(nc) as tc:
        with (
            tc.tile_pool(name="consts", bufs=1) as consts,
            tc.tile_pool(name="land", bufs=2) as land_pool,
            tc.tile_pool(name="xland", bufs=2) as xland_pool,
            tc.tile_pool(name="netin", bufs=1) as netin_pool,
            tc.tile_pool(name="scratch", bufs=1) as scratch_pool,
            tc.tile_pool(name="netx", bufs=2) as netx_pool,
            tc.tile_pool(name="med", bufs=2) as med_pool,
            tc.tile_pool(name="med32", bufs=1) as med32_pool,
            tc.tile_pool(name="medt", bufs=2) as medt_pool,
            tc.tile_pool(name="outs", bufs=2) as out_pool,
            tc.tile_pool(name="ps_tr", bufs=2, space="PSUM") as ps_tr,
            tc.tile_pool(name="ps_mm", bufs=2, space="PSUM") as ps_mm,
        ):
            kern32 = consts.tile([F, U], f32)
            nc.sync.dma_start(out=kern32, in_=k_d[:])
            kern16 = consts.tile([F, U], f16)
            nc.scalar.copy(out=kern16[:], in_=kern32[:])
            bias_sb = consts.tile([U, 1], f32)
            nc.sync.dma_start(out=bias_sb, in_=b_d[:])
            ident16 = consts.tile([128, 128], f16)
            make_identity(nc, ident16)
            ident32 = None
            if FP32_TAIL:
                ident32 = consts.tile([128, 128], f32)
                make_identity(nc, ident32)

            # scratch is single-buffered and DVE-private (serial on DVE).
            # net_in is allocated per group from a bufs=1 pool: same memory,
            # but the pool rotation gives the ScalarE writes of group g+1
            # proper WAR edges against the layer-1 reads of group g (its only
            # consumers), so conversion overlaps the rest of the network.
            scratch = scratch_pool.tile([128, 48, T, F], f16)

            def emit_tail(base, med_buf):
                # median rows -> [chan, node] via PE transpose, fp16 matmul,
                # fused bias+relu on ScalarE, stream out. Emitted one group
                # late so the in-order ScalarE queue never blocks the next
                # group's input conversions behind DVE completion.
                tail_dt = f32 if FP32_TAIL else f16
                ident = ident32 if FP32_TAIL else ident16
                kern_t = kern32 if FP32_TAIL else kern16
                medt = medt_pool.tile([128, T, F], tail_dt)
                if FP32_TAIL:
                    med32 = med32_pool.tile([128, T, F], f32)
                    nc.scalar.copy(out=med32[:], in_=med_buf[:])
                    med_buf = med32
                for t in range(T):
                    trp = ps_tr.tile([128, 128], tail_dt)
                    nc.tensor.transpose(trp, med_buf[:, t, :], ident)
                    nc.scalar.copy(out=medt[:, t, :], in_=trp)
                medt_flat = medt[:].rearrange("p t n -> p (t n)")
                out_sb = out_pool.tile([128, GROUP], f32)
                n_chunks = -(-GROUP // 512)
                chunk = GROUP // n_chunks
                for ci in range(n_chunks):
                    off = ci * chunk
                    out_ps = ps_mm.tile([128, chunk], f32)
                    nc.tensor.matmul(
                        out_ps,
                        lhsT=kern_t[:],
                        rhs=medt_flat[:, off : off + chunk],
                        start=True,
                        stop=True,
                    )
                    nc.scalar.activation(
                        out=out_sb[:, off : off + chunk],
                        in_=out_ps,
                        func=mybir.ActivationFunctionType.Relu,
                        bias=bias_sb[:],
                        scale=1.0,
                    )
                nc.sync.dma_start(out=o_d[:, base : base + GROUP], in_=out_sb)

            med_bufs = {}
            for g in range(NGROUPS + 1):
                if g < NGROUPS:
                    base = g * GROUP
                    # self row: fp32 landing + fp16 convert (read only at the
                    # very end of the network -> double-buffered)
                    x_land = xland_pool.tile([128, T, F], f32)
                    nc.sync.dma_start(
                        out=x_land[:],
                        in_=x_d[base : base + GROUP].rearrange(
                            "(t p) c -> p t c", p=128
                        ),
                    )
                    net_x = netx_pool.tile([128, T, F], f16)
                    nc.scalar.copy(out=net_x[:], in_=x_land[:])

                    # neighbors: per 128-node subtile, fp32 landing + convert
                    net_in = netin_pool.tile([128, S, T, F], f16)
                    for t in range(T):
                        nb = base + t * 128
                        land = land_pool.tile([128, S, F], f32)
                        nc.sync.dma_start(out=land[:], in_=g_d[nb : nb + 128])
                        nc.scalar.copy(out=net_in[:, :, t, :], in_=land[:])

                    written = set()

                    def run_ap(row, step, count, for_write=False,
                               written=written, net_x=net_x, net_in=net_in):
                        assert step > 0 or count == 1
                        step = max(step, 1)
                        rows = [row + i * step for i in range(count)]
                        last = rows[-1]
                        if for_write:
                            # writes always land in scratch (never the input
                            # tile, never the x row)
                            assert all(r < 48 for r in rows)
                            if count == 1:
                                return scratch[:, row]
                            return scratch[:, row : last + 1 : step]
                        if count == 1:
                            if row == _NET_XROW:
                                return net_x[:]
                            if row < S and row not in written:
                                return net_in[:, row]
                            return scratch[:, row]
                        if rows[0] < S and rows[0] not in written:
                            assert all(r < S and r not in written for r in rows)
                            return net_in[:, row : last + 1 : step]
                        assert all((r >= S or r in written) and r < 48 for r in rows)
                        return scratch[:, row : last + 1 : step]

                    # the last batch writes the median row; route it straight
                    # into the double-buffered med_buf (scratch is reused by
                    # the next group before the tail consumes the median)
                    med_buf = med_pool.tile([128, T, F], f16)
                    batches = _NET_BATCHES if NET_LIMIT is None else _NET_BATCHES[:NET_LIMIT]
                    for bi, (is_max, a, da, b, db, d, dd, cnt) in enumerate(batches):
                        final = bi == len(_NET_BATCHES) - 1
                        assert not final or (cnt == 1 and d == _NET_OUTROW)
                        out_ap = med_buf[:] if final else run_ap(d, dd, cnt, for_write=True)
                        in0 = run_ap(a, da, cnt)
                        in1 = run_ap(b, db, cnt)
                        nc.vector.tensor_tensor(
                            out=out_ap,
                            in0=in0,
                            in1=in1,
                            op=mybir.AluOpType.max if is_max else mybir.AluOpType.min,
                        )
                        written.update(d + i * dd for i in range(cnt))

                    if DEBUG_SRC is not None:
                        kind = DEBUG_SRC[0]
                        src = {"in": lambda: net_in[:, DEBUG_SRC[1]],
                               "x": lambda: net_x[:],
                               "scratch": lambda: scratch[:, DEBUG_SRC[1]]}[kind]()
                        nc.vector.tensor_copy(out=med_buf[:], in_=src)
                    elif DEBUG_ROW is not None:
                        src = net_x[:] if DEBUG_ROW == _NET_XROW else net_in[:, DEBUG_ROW]
                        nc.vector.tensor_copy(out=med_buf[:], in_=src)
                    med_bufs[g] = med_buf
                if g >= 1:
                    emit_tail((g - 1) * GROUP, med_bufs.pop(g - 1))
    nc.finalize()
    return nc


def kernel(x, neigh_x, kernel, bias):
    from concourse.bass_utils import run_bass_kernel_spmd

    x = np.ascontiguousarray(x, dtype=np.float32)
    neigh_x = np.ascontiguousarray(neigh_x, dtype=np.float32)
    kern = np.ascontiguousarray(kernel, dtype=np.float32)
    bias2 = np.ascontiguousarray(bias, dtype=np.float32).reshape(U, 1)

    per = N // NCORES
    x_pad = np.zeros((NCORES, NC_NODES, F), dtype=np.float32)
    g_pad = np.zeros((NCORES, NC_NODES, S, F), dtype=np.float32)
    for c in range(NCORES):
        x_pad[c, :per] = x[c * per : (c + 1) * per]
        g_pad[c, :per] = neigh_x[c * per : (c + 1) * per]

    nc = _build_program()
    in_maps = [
        {"x": x_pad[c], "neigh": g_pad[c], "kern": kern, "bias": bias2}
        for c in range(NCORES)
    ]
    res = run_bass_kernel_spmd(nc, in_maps, list(range(NCORES)))
    out = np.empty((N, U), dtype=np.float32)
    for c in range(NCORES):
        out[c * per : (c + 1) * per] = res.results[c]["out"][:, :per].T
    return out
